# revision 19
# baseline (speedup 1.0000x reference)
"""Multi-head attention forward (B=2, S=2048, D=1024, H=16) on 8 Trainium2
NeuronCores, tensor-parallel over heads (2 heads per core).

v4 — baseline structure (separate PSUM pools; proven scheduler behavior)
with surgical wins:
  - bk dropped entirely (softmax is invariant per-q-row:
    softmax((q+bq)@(k+bk)^T) == softmax((q+bq)@k^T) row-wise); bv folded
    into the host-side bias (sum_k p_k (v+bv) = ctx + bv since sum p = 1):
    host adds bo + bv @ Wo.T.  Saves two bias DMAs + 16 DVE bias-adds.
  - output partials in bf16 (halves output DMA); host sums in f32.
  - out_proj(1) interleaved into attention(1)'s h1 pass per q-chunk to
    pipeline the tail instead of serializing it.
  - optional (K_CTX_FP8=1): probs + augmented-V in fp8e4m3 and probs@V in
    DoubleRow perf mode (half the PE passes for ctx); scores stay f32r so
    only the softmax weights are quantized.
"""
import sys
import os

sys.path.insert(0, '/opt/trn_rl_repo')

import numpy as np
import concourse.bass as bass
import concourse.mybir as mybir
import concourse.tile as tile
from concourse import bacc, bass_utils
from concourse.masks import make_identity
import contextlib

f32 = mybir.dt.float32
f32r = mybir.dt.float32r
bf16 = mybir.dt.bfloat16
fp8 = mybir.dt.float8e4
EXP = mybir.ActivationFunctionType.Exp
DR = mybir.MatmulPerfMode.DoubleRow

B, S, D, H, HD = 2, 2048, 1024, 16, 64
T = B * S              # 4096 tokens
DC = 128               # dims per core (2 heads)
KT = 8                 # feature k-tiles (D / 128)
NCH = 8                # projection chunks of 512 tokens
NKT = 16               # k-token tiles per batch (S / 128)
NQC = 4                # q chunks of 512 per (b, h)

CTX_FP8 = os.environ.get("K_CTX_FP8", "1") == "1"


def _build():
    nc = bacc.Bacc("TRN2", target_bir_lowering=False, debug=False)
    pdt = fp8 if CTX_FP8 else f32r
    xT_d = nc.dram_tensor("xT", [D, T], f32, kind="ExternalInput").ap()
    wqT_d = nc.dram_tensor("wqT", [D, DC], f32, kind="ExternalInput").ap()
    wkT_d = nc.dram_tensor("wkT", [D, DC], f32, kind="ExternalInput").ap()
    wvT_d = nc.dram_tensor("wvT", [D, DC], f32, kind="ExternalInput").ap()
    woT_d = nc.dram_tensor("woT", [DC, D], f32, kind="ExternalInput").ap()
    bq_d = nc.dram_tensor("bq", [DC, 1], f32, kind="ExternalInput").ap()
    out_d = nc.dram_tensor("out", [T, D], bf16, kind="ExternalOutput").ap()

    xT_ap = xT_d.rearrange("(kt p) t -> p kt t", p=128)

    with tile.TileContext(nc) as tc:
        ctx = contextlib.ExitStack()
        cpool = ctx.enter_context(tc.tile_pool(name="cpool", bufs=1))
        xpool = ctx.enter_context(tc.tile_pool(name="xpool", bufs=2))
        ppool = ctx.enter_context(tc.tile_pool(name="ppool", bufs=6))
        npool = ctx.enter_context(tc.tile_pool(name="npool", bufs=2))
        opool = ctx.enter_context(tc.tile_pool(name="opool", bufs=3))
        pj = ctx.enter_context(tc.tile_pool(name="pj", bufs=2, space="PSUM"))
        sc = ctx.enter_context(tc.tile_pool(name="sc", bufs=2, space="PSUM"))
        cx = ctx.enter_context(tc.tile_pool(name="cx", bufs=2, space="PSUM"))

        # ---- constants / persistent tiles ----
        wqr = cpool.tile([128, KT, DC], f32r, tag="wqr")
        wkr = cpool.tile([128, KT, DC], f32r, tag="wkr")
        wvr = cpool.tile([128, KT, DC], f32r, tag="wvr")
        nc.gpsimd.dma_start(wqr[:], wqT_d.rearrange("(kt p) m -> p kt m", p=128))
        nc.gpsimd.dma_start(wkr[:], wkT_d.rearrange("(kt p) m -> p kt m", p=128))
        nc.gpsimd.dma_start(wvr[:], wvT_d.rearrange("(kt p) m -> p kt m", p=128))
        wor = cpool.tile([128, D], f32r, tag="wor")
        nc.gpsimd.dma_start(wor[:], woT_d[:])
        bq = cpool.tile([DC, 1], f32, tag="bq")
        nc.sync.dma_start(bq[:], bq_d[:])

        ident = cpool.tile([128, 128], f32, tag="ident")
        make_identity(nc, ident[:])
        ones = cpool.tile([128, 64], f32, tag="ones")
        nc.vector.memset(ones[:], 1.0)
        onesr = cpool.tile([128, 64], f32r, tag="onesr")
        nc.vector.tensor_copy(onesr[:], ones[:])

        # aug[p, b, h, kt, :]: augmented-V stationary per (batch, head,
        # k-token-tile).  h0: v dims at cols 0..63, ones col 64 -> ctx rows
        # 0..63, denom row 64.  h1: ones col 0, v dims at cols 64..127 ->
        # ctx rows 64..127, denom row 0.  kt slabs contiguous per head give
        # the DoubleRow pair stride.
        aug = cpool.tile([128, B, 2, NKT, 128], pdt, tag="aug")
        nc.vector.memset(aug[:].bitcast(f32 if pdt == f32r else pdt), 0.0)
        nc.vector.memset(aug[:, :, 0, :, 64:65].bitcast(f32 if pdt == f32r else pdt), 1.0)
        nc.vector.memset(aug[:, :, 1, :, 0:1].bitcast(f32 if pdt == f32r else pdt), 1.0)

        qTr = cpool.tile([128, T], f32r, tag="qTr")
        kTr = cpool.tile([128, T], f32r, tag="kTr")
        vTs = cpool.tile([128, T], f32, tag="vTs")
        ctxT = [cpool.tile([128, S], f32r, tag=f"ctxT{b}", name=f"ctxT{b}")
                for b in range(B)]

        # ---- phase 1: projections + v transposes ----
        def proj_chunk(ch):
            csl = slice(ch * 512, (ch + 1) * 512)
            xTr = xpool.tile([128, KT, 512], f32r, tag="xTr")
            if ch == 0:
                # split the first chunk's load per feature tile so the first
                # projection matmuls can start as soon as f=0 lands
                for f in range(KT):
                    nc.gpsimd.dma_start(xTr[:, f], xT_ap[:, f, csl])
            else:
                nc.gpsimd.dma_start(xTr[:], xT_ap[:, :, csl])
            for wr, dst, biased in ((wqr, qTr, True), (wkr, kTr, False),
                                    (wvr, vTs, False)):
                pp = pj.tile([128, 512], f32, tag="pj")
                for f in range(KT):
                    nc.tensor.matmul(pp[:], wr[:, f], xTr[:, f],
                                     start=(f == 0), stop=(f == KT - 1))
                if biased:
                    nc.vector.tensor_scalar_add(dst[:, csl], pp[:], bq[:])
                else:
                    nc.vector.tensor_copy(dst[:, csl], pp[:])
            vtp = pj.tile([128, 512], f32, tag="pj")
            for j in range(4):
                nc.tensor.matmul(vtp[:, j * 128:(j + 1) * 128],
                                 vTs[:, (ch * 4 + j) * 128:(ch * 4 + j + 1) * 128],
                                 ident[:], is_transpose=True,
                                 start=(j == 0), stop=(j == 3))
            for j in range(4):
                tt = ch * 4 + j
                b, kt = tt // NKT, tt % NKT
                nc.vector.tensor_copy(aug[:, b, 0, kt, 0:64],
                                      vtp[:, j * 128:j * 128 + 64])
                nc.vector.tensor_copy(aug[:, b, 1, kt, 64:128],
                                      vtp[:, j * 128 + 64:(j + 1) * 128])

        # ---- output projection for one (batch, q-chunk) ----
        def out_proj_qc(b, qc):
            for j in range(4):
                tt = qc * 4 + j
                ost = opool.tile([128, D], bf16, tag="ost", name="ost")
                for oc in range(2):
                    op = pj.tile([128, 512], f32, tag="pj", name="op")
                    nc.tensor.matmul(op[:], ctxT[b][:, tt * 128:(tt + 1) * 128],
                                     wor[:, oc * 512:(oc + 1) * 512],
                                     start=True, stop=True)
                    nc.vector.tensor_copy(ost[:, oc * 512:(oc + 1) * 512], op[:])
                nc.sync.dma_start(
                    out_d[b * S + tt * 128:b * S + (tt + 1) * 128, :], ost[:])

        # ---- phase 2/3: attention ----
        def attention(b, interleave_out=False):
            for h in range(2):
                hs = slice(h * 64, (h + 1) * 64)
                for qc in range(NQC):
                    qsl = slice(b * S + qc * 512, b * S + (qc + 1) * 512)
                    osl = slice(qc * 512, (qc + 1) * 512)
                    ctxp = cx.tile([128, 512], f32, tag="cx", name="ctxp")
                    # k-tile pairs: two kt's scoresT share one 2-bank psum
                    # tile so a single 1024-wide exp covers both
                    for kp in range(NKT // 2):
                        scp = sc.tile([128, 2, 512], f32, tag="sc", name="scp")
                        probs = ppool.tile([128, 2, 512], pdt, tag="pb",
                                           name="probs")
                        for j in range(2):
                            kt = kp * 2 + j
                            ksl = slice((b * NKT + kt) * 128,
                                        (b * NKT + kt + 1) * 128)
                            nc.tensor.matmul(scp[:, j], kTr[hs, ksl],
                                             qTr[hs, qsl],
                                             start=True, stop=True)
                        nc.scalar.activation(probs[:], scp[:], EXP, scale=0.125)
                        if CTX_FP8:
                            nc.tensor.matmul(ctxp[:],
                                             aug[:, b, h, kp * 2:kp * 2 + 2, :],
                                             probs[:], start=(kp == 0),
                                             stop=(kp == NKT // 2 - 1),
                                             perf_mode=DR)
                        else:
                            for j in range(2):
                                kt = kp * 2 + j
                                nc.tensor.matmul(ctxp[:], aug[:, b, h, kt, :],
                                                 probs[:, j], start=(kt == 0),
                                                 stop=(kt == NKT - 1))
                    if h == 0:
                        # denom at psum row 64; ctx rows 0..63
                        srow = npool.tile([128, 512], f32r, tag="srow")
                        nc.vector.tensor_copy(srow[64:65, :], ctxp[64:65, :])
                        bcp = cx.tile([128, 512], f32, tag="cx", name="bcp")
                        nc.tensor.matmul(bcp[0:64, :], onesr[64:65, 0:64],
                                         srow[64:65, :], start=True, stop=True)
                        bcs = npool.tile([128, 512], f32, tag="bcs")
                        nc.vector.reciprocal_approx_fast(bcs[0:64, :],
                                                         bcp[0:64, :])
                        nc.vector.tensor_mul(ctxT[b][0:64, osl], ctxp[0:64, :],
                                             bcs[0:64, :])
                    else:
                        # denom at psum row 0; ctx rows 64..127
                        rec = npool.tile([128, 512], f32, tag="rec")
                        nc.vector.reciprocal_approx_fast(rec[0:1, :],
                                                         ctxp[0:1, :])
                        bcp = cx.tile([128, 512], f32, tag="cx", name="bcp")
                        nc.tensor.matmul(bcp[64:128, :], ones[0:1, 0:64],
                                         rec[0:1, :], start=True, stop=True)
                        cst = npool.tile([128, 512], f32, tag="cst")
                        nc.vector.tensor_copy(cst[64:128, :], ctxp[64:128, :])
                        nc.vector.tensor_mul(ctxT[b][64:128, osl],
                                             cst[64:128, :], bcp[64:128, :])
                        if interleave_out:
                            out_proj_qc(b, qc)

        # Emission order = scheduler priority.  proj(4..7) and out_proj(0)
        # are dependency-free during the attention phases, so the list
        # scheduler slots them into PE stalls where attention waits on exp.
        for ch in range(NCH // 2):
            proj_chunk(ch)
        attention(0)
        for ch in range(NCH // 2, NCH):
            proj_chunk(ch)
        for qc in range(NQC):
            out_proj_qc(0, qc)
        attention(1, interleave_out=True)
        ctx.close()

    nc.compile()
    return nc


_NC = None


def _prep_in_maps(inputs, Wq, bq, Wk, Wv, Wo):
    x = np.ascontiguousarray(np.asarray(inputs, dtype=np.float32).reshape(T, D))
    xT = np.ascontiguousarray(x.T)
    Wq = np.asarray(Wq, dtype=np.float32)
    Wk = np.asarray(Wk, dtype=np.float32)
    Wv = np.asarray(Wv, dtype=np.float32)
    Wo = np.asarray(Wo, dtype=np.float32)

    in_maps = []
    for c in range(8):
        sl = slice(c * DC, (c + 1) * DC)
        in_maps.append({
            "xT": xT,
            "wqT": np.ascontiguousarray(Wq[sl].T),
            "wkT": np.ascontiguousarray(Wk[sl].T),
            "wvT": np.ascontiguousarray(Wv[sl].T),
            "woT": np.ascontiguousarray(Wo[:, sl].T),
            "bq": np.ascontiguousarray(np.asarray(bq, np.float32)[sl][:, None]),
        })
    return in_maps


def kernel(inputs, Wq, bq, Wk, bk, Wv, bv, Wo, bo):
    global _NC
    if _NC is None:
        _NC = _build()

    in_maps = _prep_in_maps(inputs, Wq, bq, Wk, Wv, Wo)
    res = bass_utils.run_bass_kernel_spmd(_NC, in_maps, core_ids=list(range(8)))
    out = res.results[0]["out"].astype(np.float32)
    for r in res.results[1:]:
        out += r["out"].astype(np.float32)
    # bk cancels in softmax; bv contributes bv @ Wo.T to every token
    out += (np.asarray(bo, np.float32)
            + np.asarray(bv, np.float32) @ np.asarray(Wo, np.float32).T)[None, :]
    return out.reshape(B, S, D)


# revision 24
# speedup vs baseline: 1.1130x; 1.1130x over previous
"""Multi-head attention forward (B=2, S=2048, D=1024, H=16) on 8 Trainium2
NeuronCores, tensor-parallel over heads (2 heads per core).

v4 — baseline structure (separate PSUM pools; proven scheduler behavior)
with surgical wins:
  - bk dropped entirely (softmax is invariant per-q-row:
    softmax((q+bq)@(k+bk)^T) == softmax((q+bq)@k^T) row-wise); bv folded
    into the host-side bias (sum_k p_k (v+bv) = ctx + bv since sum p = 1):
    host adds bo + bv @ Wo.T.  Saves two bias DMAs + 16 DVE bias-adds.
  - output partials in bf16 (halves output DMA); host sums in f32.
  - out_proj(1) interleaved into attention(1)'s h1 pass per q-chunk to
    pipeline the tail instead of serializing it.
  - optional (K_CTX_FP8=1): probs + augmented-V in fp8e4m3 and probs@V in
    DoubleRow perf mode (half the PE passes for ctx); scores stay f32r so
    only the softmax weights are quantized.
"""
import sys
import os

sys.path.insert(0, '/opt/trn_rl_repo')

import numpy as np
import concourse.bass as bass
import concourse.mybir as mybir
import concourse.tile as tile
from concourse import bacc, bass_utils
from concourse.masks import make_identity
import contextlib

f32 = mybir.dt.float32
f32r = mybir.dt.float32r
bf16 = mybir.dt.bfloat16
fp8 = mybir.dt.float8e4
EXP = mybir.ActivationFunctionType.Exp
DR = mybir.MatmulPerfMode.DoubleRow

B, S, D, H, HD = 2, 2048, 1024, 16, 64
T = B * S              # 4096 tokens
DC = 128               # dims per core (2 heads)
KT = 8                 # feature k-tiles (D / 128)
NCH = 8                # projection chunks of 512 tokens
NKT = 16               # k-token tiles per batch (S / 128)
NQC = 4                # q chunks of 512 per (b, h)

CTX_FP8 = os.environ.get("K_CTX_FP8", "1") == "1"


def _build():
    nc = bacc.Bacc("TRN2", target_bir_lowering=False, debug=False)
    pdt = fp8 if CTX_FP8 else f32r
    xT_d = nc.dram_tensor("xT", [D, T], f32, kind="ExternalInput").ap()
    wqT_d = nc.dram_tensor("wqT", [D, DC], f32, kind="ExternalInput").ap()
    wkT_d = nc.dram_tensor("wkT", [D, DC], f32, kind="ExternalInput").ap()
    wvT_d = nc.dram_tensor("wvT", [D, DC], f32, kind="ExternalInput").ap()
    woT_d = nc.dram_tensor("woT", [DC, D], f32, kind="ExternalInput").ap()
    bq_d = nc.dram_tensor("bq", [DC, 1], f32, kind="ExternalInput").ap()
    out_d = nc.dram_tensor("out", [T, D], bf16, kind="ExternalOutput").ap()

    xT_ap = xT_d.rearrange("(kt p) t -> p kt t", p=128)

    with tile.TileContext(nc) as tc:
        ctx = contextlib.ExitStack()
        cpool = ctx.enter_context(tc.tile_pool(name="cpool", bufs=1))
        xpool = ctx.enter_context(tc.tile_pool(name="xpool", bufs=2))
        ppool = ctx.enter_context(tc.tile_pool(name="ppool", bufs=6))
        npool = ctx.enter_context(tc.tile_pool(name="npool", bufs=2))
        opool = ctx.enter_context(tc.tile_pool(name="opool", bufs=3))
        pj = ctx.enter_context(tc.tile_pool(name="pj", bufs=2, space="PSUM"))
        sc = ctx.enter_context(tc.tile_pool(name="sc", bufs=2, space="PSUM"))
        cx = ctx.enter_context(tc.tile_pool(name="cx", bufs=2, space="PSUM"))

        # ---- constants / persistent tiles ----
        wqr = cpool.tile([128, KT, DC], f32r, tag="wqr")
        wkr = cpool.tile([128, KT, DC], f32r, tag="wkr")
        wvr = cpool.tile([128, KT, DC], f32r, tag="wvr")
        nc.gpsimd.dma_start(wqr[:], wqT_d.rearrange("(kt p) m -> p kt m", p=128))
        nc.gpsimd.dma_start(wkr[:], wkT_d.rearrange("(kt p) m -> p kt m", p=128))
        nc.gpsimd.dma_start(wvr[:], wvT_d.rearrange("(kt p) m -> p kt m", p=128))
        wor = cpool.tile([128, D], f32r, tag="wor")
        nc.gpsimd.dma_start(wor[:], woT_d[:])
        bq = cpool.tile([DC, 1], f32, tag="bq")
        nc.sync.dma_start(bq[:], bq_d[:])

        ident = cpool.tile([128, 128], f32, tag="ident")
        make_identity(nc, ident[:])
        ones = cpool.tile([128, 64], f32, tag="ones")
        nc.vector.memset(ones[:], 1.0)
        onesr = cpool.tile([128, 64], f32r, tag="onesr")
        nc.vector.tensor_copy(onesr[:], ones[:])

        # aug[p, b, h, kt, :]: augmented-V stationary per (batch, head,
        # k-token-tile).  h0: v dims at cols 0..63, ones col 64 -> ctx rows
        # 0..63, denom row 64.  h1: ones col 0, v dims at cols 64..127 ->
        # ctx rows 64..127, denom row 0.  kt slabs contiguous per head give
        # the DoubleRow pair stride.
        aug = cpool.tile([128, B, 2, NKT, 128], pdt, tag="aug")
        nc.vector.memset(aug[:].bitcast(f32 if pdt == f32r else pdt), 0.0)
        nc.vector.memset(aug[:, :, 0, :, 64:65].bitcast(f32 if pdt == f32r else pdt), 1.0)
        nc.vector.memset(aug[:, :, 1, :, 0:1].bitcast(f32 if pdt == f32r else pdt), 1.0)

        qTr = cpool.tile([128, T], f32r, tag="qTr")
        kTr = cpool.tile([128, T], f32r, tag="kTr")
        vTs = cpool.tile([128, T], f32, tag="vTs")
        ctxT = [cpool.tile([128, S], f32r, tag=f"ctxT{b}", name=f"ctxT{b}")
                for b in range(B)]

        # ---- phase 1: projections + v transposes ----
        def proj_chunk(ch):
            csl = slice(ch * 512, (ch + 1) * 512)
            xTr = xpool.tile([128, KT, 512], f32r, tag="xTr")
            if ch == 0:
                # split the first chunk's load per feature tile so the first
                # projection matmuls can start as soon as f=0 lands
                for f in range(KT):
                    nc.gpsimd.dma_start(xTr[:, f], xT_ap[:, f, csl])
            else:
                nc.gpsimd.dma_start(xTr[:], xT_ap[:, :, csl])
            for wr, dst, biased in ((wqr, qTr, True), (wkr, kTr, False),
                                    (wvr, vTs, False)):
                pp = pj.tile([128, 512], f32, tag="pj")
                for f in range(KT):
                    nc.tensor.matmul(pp[:], wr[:, f], xTr[:, f],
                                     start=(f == 0), stop=(f == KT - 1))
                if biased:
                    nc.vector.tensor_scalar_add(dst[:, csl], pp[:], bq[:])
                else:
                    nc.vector.tensor_copy(dst[:, csl], pp[:])
            vtp = pj.tile([128, 512], f32, tag="pj")
            for j in range(4):
                nc.tensor.matmul(vtp[:, j * 128:(j + 1) * 128],
                                 vTs[:, (ch * 4 + j) * 128:(ch * 4 + j + 1) * 128],
                                 ident[:], is_transpose=True,
                                 start=(j == 0), stop=(j == 3))
            for j in range(4):
                tt = ch * 4 + j
                b, kt = tt // NKT, tt % NKT
                nc.vector.tensor_copy(aug[:, b, 0, kt, 0:64],
                                      vtp[:, j * 128:j * 128 + 64])
                nc.vector.tensor_copy(aug[:, b, 1, kt, 64:128],
                                      vtp[:, j * 128 + 64:(j + 1) * 128])

        # ---- output projection for one (batch, q-chunk) ----
        def out_proj_qc(b, qc, tail=False):
            for j in range(4):
                tt = qc * 4 + j
                ost = opool.tile([128, D], bf16, tag="ost", name="ost")
                for oc in range(2):
                    op = pj.tile([128, 512], f32, tag="pj", name="op")
                    nc.tensor.matmul(op[:], ctxT[b][:, tt * 128:(tt + 1) * 128],
                                     wor[:, oc * 512:(oc + 1) * 512],
                                     start=True, stop=True)
                    # exp is done by the tail — stage through the idle ScalarE
                    if tail:
                        nc.scalar.copy(ost[:, oc * 512:(oc + 1) * 512], op[:])
                    else:
                        nc.vector.tensor_copy(ost[:, oc * 512:(oc + 1) * 512],
                                              op[:])
                nc.sync.dma_start(
                    out_d[b * S + tt * 128:b * S + (tt + 1) * 128, :], ost[:])

        # ---- phase 2/3: attention ----
        def attention(b, interleave_b0_out=False):
            for h in range(2):
                hs = slice(h * 64, (h + 1) * 64)
                for qc in range(NQC):
                    qsl = slice(b * S + qc * 512, b * S + (qc + 1) * 512)
                    osl = slice(qc * 512, (qc + 1) * 512)
                    ctxp = cx.tile([128, 512], f32, tag="cx", name="ctxp")
                    # k-tile pairs: two kt's scoresT share one 2-bank psum
                    # tile so a single 1024-wide exp covers both
                    for kp in range(NKT // 2):
                        scp = sc.tile([128, 2, 512], f32, tag="sc", name="scp")
                        probs = ppool.tile([128, 2, 512], pdt, tag="pb",
                                           name="probs")
                        for j in range(2):
                            kt = kp * 2 + j
                            ksl = slice((b * NKT + kt) * 128,
                                        (b * NKT + kt + 1) * 128)
                            nc.tensor.matmul(scp[:, j], kTr[hs, ksl],
                                             qTr[hs, qsl],
                                             start=True, stop=True)
                        nc.scalar.activation(probs[:], scp[:], EXP, scale=0.125)
                        if CTX_FP8:
                            nc.tensor.matmul(ctxp[:],
                                             aug[:, b, h, kp * 2:kp * 2 + 2, :],
                                             probs[:], start=(kp == 0),
                                             stop=(kp == NKT // 2 - 1),
                                             perf_mode=DR)
                        else:
                            for j in range(2):
                                kt = kp * 2 + j
                                nc.tensor.matmul(ctxp[:], aug[:, b, h, kt, :],
                                                 probs[:, j], start=(kt == 0),
                                                 stop=(kt == NKT - 1))
                    if h == 0:
                        # denom at psum row 64; ctx rows 0..63
                        srow = npool.tile([128, 512], f32r, tag="srow")
                        nc.vector.tensor_copy(srow[64:65, :], ctxp[64:65, :])
                        bcp = cx.tile([128, 512], f32, tag="cx", name="bcp")
                        nc.tensor.matmul(bcp[0:64, :], onesr[64:65, 0:64],
                                         srow[64:65, :], start=True, stop=True)
                        bcs = npool.tile([128, 512], f32, tag="bcs")
                        nc.vector.reciprocal_approx_fast(bcs[0:64, :],
                                                         bcp[0:64, :])
                        nc.vector.tensor_mul(ctxT[b][0:64, osl], ctxp[0:64, :],
                                             bcs[0:64, :])
                        if interleave_b0_out:
                            out_proj_qc(0, qc)
                    else:
                        # denom at psum row 0; ctx rows 64..127
                        rec = npool.tile([128, 512], f32, tag="rec")
                        nc.vector.reciprocal_approx_fast(rec[0:1, :],
                                                         ctxp[0:1, :])
                        bcp = cx.tile([128, 512], f32, tag="cx", name="bcp")
                        nc.tensor.matmul(bcp[64:128, :], ones[0:1, 0:64],
                                         rec[0:1, :], start=True, stop=True)
                        cst = npool.tile([128, 512], f32, tag="cst")
                        nc.vector.tensor_copy(cst[64:128, :], ctxp[64:128, :])
                        nc.vector.tensor_mul(ctxT[b][64:128, osl],
                                             cst[64:128, :], bcp[64:128, :])
                        if interleave_b0_out:
                            out_proj_qc(1, qc, tail=(qc == NQC - 1))

        # Emission order = scheduler priority.  proj(4..7) fills attention(0)
        # exp stalls; out_proj(0)/out_proj(1) are spread per q-chunk through
        # attention(1)'s h0/h1 passes so the PE filler work is evenly
        # distributed instead of starving the exp pipeline in one block.
        for ch in range(NCH // 2):
            proj_chunk(ch)
        attention(0)
        for ch in range(NCH // 2, NCH):
            proj_chunk(ch)
        attention(1, interleave_b0_out=True)
        ctx.close()

    nc.compile()
    return nc


_NC = None


def _prep_in_maps(inputs, Wq, bq, Wk, Wv, Wo):
    x = np.ascontiguousarray(np.asarray(inputs, dtype=np.float32).reshape(T, D))
    xT = np.ascontiguousarray(x.T)
    Wq = np.asarray(Wq, dtype=np.float32)
    Wk = np.asarray(Wk, dtype=np.float32)
    Wv = np.asarray(Wv, dtype=np.float32)
    Wo = np.asarray(Wo, dtype=np.float32)

    in_maps = []
    for c in range(8):
        sl = slice(c * DC, (c + 1) * DC)
        in_maps.append({
            "xT": xT,
            "wqT": np.ascontiguousarray(Wq[sl].T),
            "wkT": np.ascontiguousarray(Wk[sl].T),
            "wvT": np.ascontiguousarray(Wv[sl].T),
            "woT": np.ascontiguousarray(Wo[:, sl].T),
            "bq": np.ascontiguousarray(np.asarray(bq, np.float32)[sl][:, None]),
        })
    return in_maps


def kernel(inputs, Wq, bq, Wk, bk, Wv, bv, Wo, bo):
    global _NC
    if _NC is None:
        _NC = _build()

    in_maps = _prep_in_maps(inputs, Wq, bq, Wk, Wv, Wo)
    res = bass_utils.run_bass_kernel_spmd(_NC, in_maps, core_ids=list(range(8)))
    out = res.results[0]["out"].astype(np.float32)
    for r in res.results[1:]:
        out += r["out"].astype(np.float32)
    # bk cancels in softmax; bv contributes bv @ Wo.T to every token
    out += (np.asarray(bo, np.float32)
            + np.asarray(bv, np.float32) @ np.asarray(Wo, np.float32).T)[None, :]
    return out.reshape(B, S, D)
